# revision 1
# baseline (speedup 1.0000x reference)
"""DescriptorDiversityLoss on 8 Trainium2 NeuronCores.

Reference computes sim = F F^T (M x M, M = 8192) and returns
|(sum(sim) - trace(sim)) / (M^2 - M)|.

Math identity used here (exact in real arithmetic):
    sum(sim)   = || sum_i f_i ||^2           (f_i = rows of F)
    trace(sim) = sum_i ||f_i||^2             (total sum of squares)
so the whole loss is a single pass over the 8 MiB input: per core we
need (a) the column sums of its row block and (b) the total sum of
squares of its row block.  Host combines the 8 partials.

Sharding: rows of desc_flat split evenly across 8 cores (1024 rows,
1 MiB each).  Per core the (1024, 256) block is viewed as (128, 2048)
(partition p holds rows 8p..8p+7) and processed in 4 chunks of
(128, 512):
  - TensorE: ones(128,1)^T @ chunk accumulated in PSUM -> (1, 512)
    column sums (the two 256-halves are folded on host).
  - ScalarE: activation(Square) with accum_out -> per-partition sum of
    squares of the chunk, one column of a (128, 4) tile; a final tiny
    matmul with the ones vector reduces it over partitions -> (1, 4).
"""

import numpy as np

import concourse.bass as bass
import concourse.bacc as bacc
import concourse.mybir as mybir
import concourse.tile as tile
from concourse.bass_utils import run_bass_kernel_spmd

B, N, D = 16, 512, 256
M = B * N                 # 8192 descriptors total
N_CORES = 8
ROWS = M // N_CORES       # 1024 rows per core
P = 128                   # SBUF partitions
FREE = ROWS * D // P      # 2048 f32 per partition (8 KiB contiguous)
N_CH = 4
CH = FREE // N_CH         # 512 (one PSUM bank of f32)
OUT_W = CH + N_CH         # 516: [0:512] col sums, [512:516] sumsq parts

_cached_nc = None


def _build_nc():
    f32 = mybir.dt.float32
    nc = bacc.Bacc("TRN2", target_bir_lowering=False, debug=False)
    x = nc.dram_tensor("x", [P, FREE], f32, kind="ExternalInput")
    out = nc.dram_tensor("out", [1, OUT_W], f32, kind="ExternalOutput")

    with tile.TileContext(nc) as tc:
        with (
            tc.tile_pool(name="const", bufs=1) as cpool,
            tc.tile_pool(name="inp", bufs=N_CH) as ipool,
            tc.tile_pool(name="sq", bufs=2) as qpool,
            tc.tile_pool(name="acc", bufs=1) as apool,
            tc.tile_pool(name="psum", bufs=1, space=bass.MemorySpace.PSUM) as ppool,
            tc.tile_pool(name="outp", bufs=1) as opool,
        ):
            ones = cpool.tile([P, 1], f32)
            nc.vector.memset(ones[:], 1.0)
            rowsq = apool.tile([P, N_CH], f32)
            ps = ppool.tile([1, CH], f32)
            pt = ppool.tile([1, N_CH], f32)

            for j in range(N_CH):
                t = ipool.tile([P, CH], f32)
                nc.sync.dma_start(t[:], x[:, j * CH:(j + 1) * CH])
                nc.tensor.matmul(
                    ps[:], ones[:], t[:], start=(j == 0), stop=(j == N_CH - 1)
                )
                sq = qpool.tile([P, CH], f32)
                nc.scalar.activation(
                    sq[:],
                    t[:],
                    mybir.ActivationFunctionType.Square,
                    accum_out=rowsq[:, j:j + 1],
                )

            nc.tensor.matmul(pt[:], ones[:], rowsq[:], start=True, stop=True)

            o = opool.tile([1, OUT_W], f32)
            nc.vector.tensor_copy(o[:, :CH], ps[:])
            nc.vector.tensor_copy(o[:, CH:], pt[:])
            nc.sync.dma_start(out[:], o[:])

    nc.compile()
    return nc


def kernel(descriptors: np.ndarray) -> np.ndarray:
    global _cached_nc
    if _cached_nc is None:
        _cached_nc = _build_nc()
    nc = _cached_nc

    flat = np.ascontiguousarray(descriptors, dtype=np.float32).reshape(M, D)
    in_maps = [
        {"x": flat[c * ROWS:(c + 1) * ROWS].reshape(P, FREE)}
        for c in range(N_CORES)
    ]
    results = run_bass_kernel_spmd(nc, in_maps, core_ids=list(range(N_CORES)))

    rs = np.stack([r["out"][0] for r in results.results]).astype(np.float64)
    s = rs[:, :D].sum(axis=0) + rs[:, D:CH].sum(axis=0)  # (256,) col sums
    sumsq = rs[:, CH:].sum()                             # trace(sim)
    off_diag = float(s @ s) - sumsq
    loss = abs(off_diag / (M * (M - 1)))
    return np.float32(loss)


# revision 3
# speedup vs baseline: 1.6410x; 1.6410x over previous
"""DescriptorDiversityLoss on 8 Trainium2 NeuronCores.

Reference computes sim = F F^T (M x M, M = 8192) and returns
|(sum(sim) - trace(sim)) / (M^2 - M)|.

Math identity used (exact in real arithmetic):
    sum(sim)   = || sum_i f_i ||^2           (f_i = rows of F)
    trace(sim) = sum_i ||f_i||^2             (total sum of squares)
so the loss needs one pass over the 8 MiB input: per core (a) column
sums of its row block and (b) its total sum of squares.

Sharding: rows split across 8 cores (1024 rows / 1 MiB each).  The
per-core (1024, 256) block is viewed as (128, 2048) - partition p holds
rows 8p..8p+7 - and loaded in 4 column-chunks of (128, 512) so compute
pipelines with the DMAs.  Column c of the view maps to original column
c % 256, so summing 256-strided column groups preserves column
identity:
  - VectorE folds each chunk's two 256-halves into a running acc
    (128, 256); the host finishes the partition/core reduction.
  - Squares for the trace split between ScalarE (activation Square,
    accum_out -> per-partition row sums) and VectorE
    (tensor_tensor_reduce) so neither engine trails the DMA stream.
Host combines the 8 (128, 264) partials: ~270 KB, trivial numpy work.
"""

import numpy as np

import concourse.bacc as bacc
import concourse.mybir as mybir
import concourse.tile as tile
from concourse.bass_utils import run_bass_kernel_spmd

B, N, D = 16, 512, 256
M = B * N                 # 8192 descriptors total
N_CORES = 8
ROWS = M // N_CORES       # 1024 rows per core
P = 128                   # SBUF partitions
FREE = ROWS * D // P      # 2048 f32 per partition (8 KiB contiguous)
N_CH = 4
CH = FREE // N_CH         # 512 columns per chunk
W_ACT = 384               # chunk width squared on ScalarE; rest on VectorE
OUT_W = D + 2 * N_CH      # 264: [0:256] col sums, [256:264] rowsq partials

_cached_nc = None


def _build_nc():
    f32 = mybir.dt.float32
    nc = bacc.Bacc("TRN2", target_bir_lowering=False, debug=False)
    x = nc.dram_tensor("x", [P, FREE], f32, kind="ExternalInput")
    out = nc.dram_tensor("out", [P, OUT_W], f32, kind="ExternalOutput")

    with tile.TileContext(nc) as tc:
        with (
            tc.tile_pool(name="inp", bufs=N_CH) as ipool,
            tc.tile_pool(name="sqa", bufs=2) as apool,
            tc.tile_pool(name="sqd", bufs=2) as dpool,
            tc.tile_pool(name="outp", bufs=1) as opool,
        ):
            o = opool.tile([P, OUT_W], f32)
            acc = o[:, :D]

            for j in range(N_CH):
                t = ipool.tile([P, CH], f32)
                nc.sync.dma_start(t[:], x[:, j * CH:(j + 1) * CH])

                # fold the chunk's two 256-column halves into acc
                if j == 0:
                    nc.vector.tensor_add(acc, t[:, :D], t[:, D:2 * D])
                else:
                    nc.vector.tensor_add(acc, acc, t[:, :D])
                    nc.vector.tensor_add(acc, acc, t[:, D:2 * D])

                # row sums of squares, split ACT / DVE
                sqa = apool.tile([P, W_ACT], f32)
                nc.scalar.activation(
                    sqa[:],
                    t[:, :W_ACT],
                    mybir.ActivationFunctionType.Square,
                    accum_out=o[:, D + j:D + j + 1],
                )
                sqd = dpool.tile([P, CH - W_ACT], f32)
                nc.vector.scalar_tensor_tensor(
                    sqd[:],
                    t[:, W_ACT:],
                    1.0,
                    t[:, W_ACT:],
                    op0=mybir.AluOpType.mult,
                    op1=mybir.AluOpType.mult,
                    accum_out=o[:, D + N_CH + j:D + N_CH + j + 1],
                )

            nc.sync.dma_start(out[:], o[:])

    nc.compile()
    return nc


def kernel(descriptors: np.ndarray) -> np.ndarray:
    global _cached_nc
    if _cached_nc is None:
        _cached_nc = _build_nc()
    nc = _cached_nc

    flat = np.ascontiguousarray(descriptors, dtype=np.float32).reshape(M, D)
    in_maps = [
        {"x": flat[c * ROWS:(c + 1) * ROWS].reshape(P, FREE)}
        for c in range(N_CORES)
    ]
    results = run_bass_kernel_spmd(nc, in_maps, core_ids=list(range(N_CORES)))

    rs = np.stack([r["out"] for r in results.results]).astype(np.float64)
    s = rs[:, :, :D].sum(axis=(0, 1))   # (256,) global column sums
    sumsq = rs[:, :, D:].sum()          # trace(sim)
    off_diag = float(s @ s) - sumsq
    loss = abs(off_diag / (M * (M - 1)))
    return np.float32(loss)


# revision 10
# speedup vs baseline: 1.6902x; 1.0299x over previous
"""DescriptorDiversityLoss on 8 Trainium2 NeuronCores.

Reference computes sim = F F^T (M x M, M = 8192) and returns
|(sum(sim) - trace(sim)) / (M^2 - M)|.

Math identity used (exact in real arithmetic):
    sum(sim)   = || sum_i f_i ||^2           (f_i = rows of F)
    trace(sim) = sum_i ||f_i||^2             (total sum of squares)
so the loss needs one pass over the 8 MiB input: per core (a) column
sums of its row block and (b) its total sum of squares.

Sharding: rows split across 8 cores (1024 rows / 1 MiB each).  The
per-core (1024, 256) block is viewed as (128, 2048) - partition p holds
rows 8p..8p+7 - and streamed in column-chunks so compute pipelines with
the DMA stream (the last chunks are smaller to shrink the post-stream
tail).  Column c of the view maps to original column c % 256, so
256-strided folds preserve column identity:
  - VectorE folds chunks into a running acc (128, 256); the host
    finishes the partition/core reduction (~270 KB total, trivial).
  - Squares for the trace are split across ScalarE (activation Square
    with accum_out), GpSimd and VectorE (scalar_tensor_tensor with
    accum_out) so no single engine trails the DMA stream; each segment
    deposits a per-partition row-sum column that the host sums.
"""

import numpy as np

import concourse.bacc as bacc
import concourse.mybir as mybir
import concourse.tile as tile
from concourse.bass_utils import run_bass_kernel_spmd

B, N, D = 16, 512, 256
M = B * N                 # 8192 descriptors total
N_CORES = 8
ROWS = M // N_CORES       # 1024 rows per core
P = 128                   # SBUF partitions
FREE = ROWS * D // P      # 2048 f32 per partition (8 KiB contiguous)

# chunk widths (cols) and per-chunk square split (act_w, pool_w, dve_w)
CFG = {
    "widths": [512, 512, 512, 512],
    "squares": [
        (512, 0, 0),
        (512, 0, 0),
        (512, 0, 0),
        (512, 0, 0),
    ],
}

_cached_nc = None


def _build_nc(cfg=CFG):
    f32 = mybir.dt.float32
    widths = cfg["widths"]
    squares = cfg["squares"]
    assert sum(widths) == FREE
    n_seg = sum(1 for sp in squares for w in sp if w > 0)
    out_w = D + n_seg

    nc = bacc.Bacc("TRN2", target_bir_lowering=False, debug=False)
    x = nc.dram_tensor("x", [P, FREE], f32, kind="ExternalInput")
    out = nc.dram_tensor("out", [P, out_w], f32, kind="ExternalOutput")

    with tile.TileContext(nc) as tc:
        with (
            tc.tile_pool(name="inp", bufs=len(widths)) as ipool,
            tc.tile_pool(name="sq", bufs=3) as qpool,
            tc.tile_pool(name="outp", bufs=1) as opool,
        ):
            o = opool.tile([P, out_w], f32)
            acc = o[:, :D]
            seg = 0          # next rowsq column
            col = 0          # running column offset into x
            first = True
            seg_kinds = []   # "full" (sum whole column) or "scalar" (row 0)
            for j, w in enumerate(widths):
                t = ipool.tile([P, w], f32, tag=f"t{j}")
                nc.sync.dma_start(t[:], x[:, col:col + w])
                col += w

                # fold the chunk's 256-col blocks into acc (VectorE)
                n_blk = w // D
                if first:
                    assert n_blk >= 2, "first chunk must have >= 2 blocks"
                    nc.vector.tensor_add(acc, t[:, :D], t[:, D:2 * D])
                    blks = range(2, n_blk)
                    first = False
                else:
                    blks = range(n_blk)
                for b in blks:
                    nc.vector.tensor_add(acc, acc, t[:, b * D:(b + 1) * D])

                # sums of squares, segmented across ACT / Pool / DVE.
                # ACT/DVE deposit per-partition row sums (full column);
                # Pool (no accum_out support) squares then full-reduces to a
                # single scalar in row 0 of its column.
                act_w, pool_w, dve_w = squares[j]
                assert act_w + pool_w + dve_w == w
                off = 0
                for eng_name, ew in (("act", act_w), ("pool", pool_w),
                                     ("dve", dve_w)):
                    if ew == 0:
                        continue
                    src = t[:, off:off + ew]
                    sq = qpool.tile([P, ew], f32, tag=f"sq{seg}")
                    accum = o[:, D + seg:D + seg + 1]
                    if eng_name == "act":
                        nc.scalar.activation(
                            sq[:], src,
                            mybir.ActivationFunctionType.Square,
                            accum_out=accum,
                        )
                        seg_kinds.append("full")
                    elif eng_name == "pool":
                        nc.gpsimd.tensor_tensor(
                            sq[:], src, src, op=mybir.AluOpType.mult
                        )
                        nc.gpsimd.tensor_reduce(
                            o[:1, D + seg:D + seg + 1], sq[:],
                            axis=mybir.AxisListType.XYZWC,
                            op=mybir.AluOpType.add,
                        )
                        seg_kinds.append("scalar")
                    else:
                        nc.vector.scalar_tensor_tensor(
                            sq[:], src, 1.0, src,
                            op0=mybir.AluOpType.mult,
                            op1=mybir.AluOpType.mult,
                            accum_out=accum,
                        )
                        seg_kinds.append("full")
                    off += ew
                    seg += 1

            nc.sync.dma_start(out[:], o[:])

    nc.compile()
    nc._out_w = out_w
    nc._seg_kinds = seg_kinds
    return nc


def kernel(descriptors: np.ndarray) -> np.ndarray:
    global _cached_nc
    if _cached_nc is None:
        _cached_nc = _build_nc()
    nc = _cached_nc

    flat = np.ascontiguousarray(descriptors, dtype=np.float32).reshape(M, D)
    in_maps = [
        {"x": flat[c * ROWS:(c + 1) * ROWS].reshape(P, FREE)}
        for c in range(N_CORES)
    ]
    results = run_bass_kernel_spmd(nc, in_maps, core_ids=list(range(N_CORES)))

    rs = np.stack([r["out"] for r in results.results]).astype(np.float64)
    s = rs[:, :, :D].sum(axis=(0, 1))   # (256,) global column sums
    sumsq = 0.0                         # trace(sim)
    for i, kind in enumerate(nc._seg_kinds):
        col = rs[:, :, D + i]
        sumsq += col.sum() if kind == "full" else col[:, 0].sum()
    off_diag = float(s @ s) - sumsq
    loss = abs(off_diag / (M * (M - 1)))
    return np.float32(loss)


# revision 12
# speedup vs baseline: 1.7466x; 1.0334x over previous
"""DescriptorDiversityLoss on 8 Trainium2 NeuronCores.

Reference computes sim = F F^T (M x M, M = 8192) and returns
|(sum(sim) - trace(sim)) / (M^2 - M)|.

Math identity used (exact in real arithmetic):
    sum(sim)   = || sum_i f_i ||^2           (f_i = rows of F)
    trace(sim) = sum_i ||f_i||^2             (total sum of squares)
so the loss needs one pass over the 8 MiB input: per core (a) column
sums of its row block and (b) its total sum of squares.

Sharding: rows split across 8 cores (1024 rows / 1 MiB each).  The
per-core (1024, 256) block is viewed as (128, 2048) - partition p holds
rows 8p..8p+7 - and streamed in column-chunks so compute pipelines with
the DMA stream (the last chunks are smaller to shrink the post-stream
tail).  Column c of the view maps to original column c % 256, so
256-strided folds preserve column identity:
  - VectorE folds chunks into a running acc (128, 256); the host
    finishes the partition/core reduction (~270 KB total, trivial).
  - Squares for the trace are split across ScalarE (activation Square
    with accum_out), GpSimd and VectorE (scalar_tensor_tensor with
    accum_out) so no single engine trails the DMA stream; each segment
    deposits a per-partition row-sum column that the host sums.
"""

import numpy as np

import concourse.bacc as bacc
import concourse.bass as cbass
import concourse.mybir as mybir
import concourse.tile as tile
from concourse.bass_utils import run_bass_kernel_spmd

B, N, D = 16, 512, 256
M = B * N                 # 8192 descriptors total
N_CORES = 8
ROWS = M // N_CORES       # 1024 rows per core
P = 128                   # SBUF partitions
FREE = ROWS * D // P      # 2048 f32 per partition (8 KiB contiguous)

# chunk widths (cols) and per-chunk square split (act_w, pool_w, dve_w)
CFG = {
    "widths": [512, 512, 512, 512],
    "squares": [
        (512, 0, 0),
        (512, 0, 0),
        (512, 0, 0),
        (512, 0, 0),
    ],
}

_cached_nc = None


def _build_nc(cfg=CFG):
    f32 = mybir.dt.float32
    widths = cfg["widths"]
    squares = cfg["squares"]
    assert sum(widths) == FREE
    n_seg = sum(1 for sp in squares for w in sp if w > 0)
    out_w = D + n_seg

    # Bass.__init__ unconditionally emits a 4-entry const bank via Pool
    # memsets, and the kernel-start barrier waits for them (~0.3 us).  Only
    # const-float32-0.0 (the Square bias) is read here: skip the other three
    # and emit the needed one on the otherwise-idle VectorE.
    orig_memset = cbass.BassGpSimd.memset

    def patched_memset(self, ap, constant):
        name = getattr(ap.tensor, "name", "")
        if name.startswith(
            ("const-float32-1.0", "const-bfloat16-1.0", "const-uint8-127")
        ):
            return None
        if name.startswith("const-float32-0.0"):
            return self.bass.vector.memset(ap, constant)
        return orig_memset(self, ap, constant)

    cbass.BassGpSimd.memset = patched_memset
    try:
        nc = bacc.Bacc("TRN2", target_bir_lowering=False, debug=False)
    finally:
        cbass.BassGpSimd.memset = orig_memset
    x = nc.dram_tensor("x", [P, FREE], f32, kind="ExternalInput")
    out = nc.dram_tensor("out", [P, out_w], f32, kind="ExternalOutput")

    with tile.TileContext(nc) as tc:
        with (
            tc.tile_pool(name="inp", bufs=len(widths)) as ipool,
            tc.tile_pool(name="sq", bufs=3) as qpool,
            tc.tile_pool(name="outp", bufs=1) as opool,
        ):
            o = opool.tile([P, out_w], f32)
            acc = o[:, :D]
            seg = 0          # next rowsq column
            col = 0          # running column offset into x
            first = True
            seg_kinds = []   # "full" (sum whole column) or "scalar" (row 0)
            for j, w in enumerate(widths):
                t = ipool.tile([P, w], f32, tag=f"t{j}")
                nc.sync.dma_start(t[:], x[:, col:col + w])
                col += w

                # fold the chunk's 256-col blocks into acc (VectorE)
                n_blk = w // D
                if first:
                    assert n_blk >= 2, "first chunk must have >= 2 blocks"
                    nc.vector.tensor_add(acc, t[:, :D], t[:, D:2 * D])
                    blks = range(2, n_blk)
                    first = False
                else:
                    blks = range(n_blk)
                for b in blks:
                    nc.vector.tensor_add(acc, acc, t[:, b * D:(b + 1) * D])

                # sums of squares, segmented across ACT / Pool / DVE.
                # ACT/DVE deposit per-partition row sums (full column);
                # Pool (no accum_out support) squares then full-reduces to a
                # single scalar in row 0 of its column.
                act_w, pool_w, dve_w = squares[j]
                assert act_w + pool_w + dve_w == w
                off = 0
                for eng_name, ew in (("act", act_w), ("pool", pool_w),
                                     ("dve", dve_w)):
                    if ew == 0:
                        continue
                    src = t[:, off:off + ew]
                    sq = qpool.tile([P, ew], f32, tag=f"sq{seg}")
                    accum = o[:, D + seg:D + seg + 1]
                    if eng_name == "act":
                        nc.scalar.activation(
                            sq[:], src,
                            mybir.ActivationFunctionType.Square,
                            accum_out=accum,
                        )
                        seg_kinds.append("full")
                    elif eng_name == "pool":
                        nc.gpsimd.tensor_tensor(
                            sq[:], src, src, op=mybir.AluOpType.mult
                        )
                        nc.gpsimd.tensor_reduce(
                            o[:1, D + seg:D + seg + 1], sq[:],
                            axis=mybir.AxisListType.XYZWC,
                            op=mybir.AluOpType.add,
                        )
                        seg_kinds.append("scalar")
                    else:
                        nc.vector.scalar_tensor_tensor(
                            sq[:], src, 1.0, src,
                            op0=mybir.AluOpType.mult,
                            op1=mybir.AluOpType.mult,
                            accum_out=accum,
                        )
                        seg_kinds.append("full")
                    off += ew
                    seg += 1

            nc.sync.dma_start(out[:], o[:])

    nc.compile()
    nc._out_w = out_w
    nc._seg_kinds = seg_kinds
    return nc


def kernel(descriptors: np.ndarray) -> np.ndarray:
    global _cached_nc
    if _cached_nc is None:
        _cached_nc = _build_nc()
    nc = _cached_nc

    flat = np.ascontiguousarray(descriptors, dtype=np.float32).reshape(M, D)
    in_maps = [
        {"x": flat[c * ROWS:(c + 1) * ROWS].reshape(P, FREE)}
        for c in range(N_CORES)
    ]
    results = run_bass_kernel_spmd(nc, in_maps, core_ids=list(range(N_CORES)))

    rs = np.stack([r["out"] for r in results.results]).astype(np.float64)
    s = rs[:, :, :D].sum(axis=(0, 1))   # (256,) global column sums
    sumsq = 0.0                         # trace(sim)
    for i, kind in enumerate(nc._seg_kinds):
        col = rs[:, :, D + i]
        sumsq += col.sum() if kind == "full" else col[:, 0].sum()
    off_diag = float(s @ s) - sumsq
    loss = abs(off_diag / (M * (M - 1)))
    return np.float32(loss)


# revision 20
# speedup vs baseline: 1.9107x; 1.0940x over previous
"""DescriptorDiversityLoss on 8 Trainium2 NeuronCores.

Reference computes sim = F F^T (M x M, M = 8192) and returns
|(sum(sim) - trace(sim)) / (M^2 - M)|.

Math identity used (exact in real arithmetic):
    sum(sim)   = || sum_i f_i ||^2           (f_i = rows of F)
    trace(sim) = sum_i ||f_i||^2             (total sum of squares)
so the loss needs one pass over the 8 MiB input: per core (a) column
sums of its row block and (b) its total sum of squares.

Sharding: rows split across 8 cores (1024 rows / 1 MiB each).  The
per-core (1024, 256) block is viewed as (128, 2048) - partition p holds
rows 8p..8p+7 - and streamed in column-chunks so compute pipelines with
the DMA stream (the last chunks are smaller to shrink the post-stream
tail).  Column c of the view maps to original column c % 256, so
256-strided folds preserve column identity:
  - VectorE folds chunks into a running acc (128, 256); the host
    finishes the partition/core reduction (~270 KB total, trivial).
  - Squares for the trace are split across ScalarE (activation Square
    with accum_out), GpSimd and VectorE (scalar_tensor_tensor with
    accum_out) so no single engine trails the DMA stream; each segment
    deposits a per-partition row-sum column that the host sums.
"""

import numpy as np

import concourse.bacc as bacc
import concourse.bass as cbass
import concourse.mybir as mybir
import concourse.tile as tile
from concourse.bass_utils import run_bass_kernel_spmd

B, N, D = 16, 512, 256
M = B * N                 # 8192 descriptors total
N_CORES = 8
ROWS = M // N_CORES       # 1024 rows per core
P = 128                   # SBUF partitions
FREE = ROWS * D // P      # 2048 f32 per partition (8 KiB contiguous)

# chunk widths (cols) and per-chunk square split (act_w, pool_w, dve_w)
CFG = {
    "widths": [512, 512, 512, 512],
    "squares": [
        (512, 0, 0),
        (512, 0, 0),
        (256, 256, 0),
        (512, 0, 0),
    ],
}


def _patched_drain_and_barrier(self, tick_clock, wait_clock):
    """Tile kernel tail minus the second all-engine barrier.

    Stock Tile emits drain -> barrier -> sem-clear -> barrier.  The final
    barrier only keeps engines from running past the sem-clears, but each
    engine's stream simply ends here and NRT waits for all engines anyway;
    the clears still complete on their issuing engine.  Dropping it saves
    ~260 ns and repeat executions stay correct (sems are still cleared).
    """
    from concourse.tile import ScopedClock

    drain_inst = self.nc.sync.drain()
    wait_clock.add_sem_waits(
        drain_inst.ins, ScopedClock({None: tick_clock.global_clock})
    )
    self.nc.all_engine_barrier()
    popped = self.nc._tile_sem_poison_stack.pop()
    assert popped is self._sem_poison
    self.nc.clear_and_free_semaphores(list(self.sems.allocated().values()))

_cached_nc = None


def _build_nc(cfg=CFG):
    f32 = mybir.dt.float32
    widths = cfg["widths"]
    squares = cfg["squares"]
    assert sum(widths) == FREE
    n_seg = sum(1 for sp in squares for w in sp if w > 0)
    out_w = D + n_seg

    # Bass.__init__ unconditionally emits a 4-entry const bank via Pool
    # memsets plus an all-engine barrier, and every engine waits on that
    # barrier before starting (~0.6 us).  Only const-float32-0.0 (the Square
    # bias) is read here - and its first reader (ScalarE, gated on the first
    # DMA chunk, ~3 us in) trails the memset by orders of magnitude - so:
    # skip the three unused consts, emit the needed one on the otherwise-idle
    # VectorE, and drop the init barrier entirely.
    orig_memset = cbass.BassGpSimd.memset
    orig_barrier = cbass.Bass.all_engine_barrier

    def patched_memset(self, ap, constant):
        name = getattr(ap.tensor, "name", "")
        if name.startswith(
            ("const-float32-1.0", "const-bfloat16-1.0", "const-uint8-127")
        ):
            return None
        if name.startswith("const-float32-0.0"):
            return self.bass.vector.memset(ap, constant)
        return orig_memset(self, ap, constant)

    cbass.BassGpSimd.memset = patched_memset
    cbass.Bass.all_engine_barrier = lambda self, *a, **k: None
    try:
        nc = bacc.Bacc("TRN2", target_bir_lowering=False, debug=False)
    finally:
        cbass.BassGpSimd.memset = orig_memset
        cbass.Bass.all_engine_barrier = orig_barrier
    x = nc.dram_tensor("x", [P, FREE], f32, kind="ExternalInput")
    out = nc.dram_tensor("out", [P, out_w], f32, kind="ExternalOutput")

    orig_dab = tile.TileContext._drain_and_barrier
    tile.TileContext._drain_and_barrier = _patched_drain_and_barrier
    try:
        _emit_tile_program(nc, widths, squares, out_w, x, out)
    finally:
        tile.TileContext._drain_and_barrier = orig_dab

    nc.compile()
    nc._out_w = out_w
    nc._seg_kinds = _seg_kinds_for(squares)
    return nc


def _seg_kinds_for(squares):
    kinds = []
    for act_w, pool_w, dve_w in squares:
        if act_w:
            kinds.append("full")
        if pool_w:
            kinds.append("scalar")
        if dve_w:
            kinds.append("full")
    return kinds


def _emit_tile_program(nc, widths, squares, out_w, x, out):
    f32 = mybir.dt.float32
    with tile.TileContext(nc) as tc:
        with (
            tc.tile_pool(name="inp", bufs=len(widths)) as ipool,
            tc.tile_pool(name="sq", bufs=3) as qpool,
            tc.tile_pool(name="ufold", bufs=2) as upool,
            tc.tile_pool(name="outp", bufs=1) as opool,
        ):
            o = opool.tile([P, out_w], f32)
            acc = o[:, :D]
            seg = 0          # next rowsq column
            col = 0          # running column offset into x
            first = True
            for j, w in enumerate(widths):
                t = ipool.tile([P, w], f32, tag=f"t{j}")
                nc.sync.dma_start(t[:], x[:, col:col + w])
                col += w

                # fold the chunk's 256-col blocks into acc (VectorE).  The
                # acc chain is latency-bound (~420ns per dependent link), so
                # a 2-block chunk first pair-folds into an independent tile
                # (no chain dependency, issues back-to-back) and merges once.
                n_blk = w // D
                if first:
                    assert n_blk >= 2, "first chunk must have >= 2 blocks"
                    nc.vector.tensor_add(acc, t[:, :D], t[:, D:2 * D])
                    for b in range(2, n_blk):
                        nc.vector.tensor_add(acc, acc, t[:, b * D:(b + 1) * D])
                    first = False
                elif n_blk == 2:
                    u = upool.tile([P, D], f32, tag=f"u{j}")
                    nc.vector.tensor_add(u[:], t[:, :D], t[:, D:2 * D])
                    nc.vector.tensor_add(acc, acc, u[:])
                else:
                    for b in range(n_blk):
                        nc.vector.tensor_add(acc, acc, t[:, b * D:(b + 1) * D])

                # sums of squares, segmented across ACT / Pool / DVE.
                # ACT/DVE deposit per-partition row sums (full column);
                # Pool (no accum_out support) squares then full-reduces to a
                # single scalar in row 0 of its column.
                act_w, pool_w, dve_w = squares[j]
                assert act_w + pool_w + dve_w == w
                off = 0
                for eng_name, ew in (("act", act_w), ("pool", pool_w),
                                     ("dve", dve_w)):
                    if ew == 0:
                        continue
                    src = t[:, off:off + ew]
                    sq = qpool.tile([P, ew], f32, tag=f"sq{seg}")
                    accum = o[:, D + seg:D + seg + 1]
                    if eng_name == "act":
                        nc.scalar.activation(
                            sq[:], src,
                            mybir.ActivationFunctionType.Square,
                            accum_out=accum,
                        )
                    elif eng_name == "pool":
                        nc.gpsimd.tensor_tensor(
                            sq[:], src, src, op=mybir.AluOpType.mult
                        )
                        nc.gpsimd.tensor_reduce(
                            o[:1, D + seg:D + seg + 1], sq[:],
                            axis=mybir.AxisListType.XYZWC,
                            op=mybir.AluOpType.add,
                        )
                    else:
                        nc.vector.scalar_tensor_tensor(
                            sq[:], src, 1.0, src,
                            op0=mybir.AluOpType.mult,
                            op1=mybir.AluOpType.mult,
                            accum_out=accum,
                        )
                    off += ew
                    seg += 1

            nc.sync.dma_start(out[:], o[:])


def kernel(descriptors: np.ndarray) -> np.ndarray:
    global _cached_nc
    if _cached_nc is None:
        _cached_nc = _build_nc()
    nc = _cached_nc

    flat = np.ascontiguousarray(descriptors, dtype=np.float32).reshape(M, D)
    in_maps = [
        {"x": flat[c * ROWS:(c + 1) * ROWS].reshape(P, FREE)}
        for c in range(N_CORES)
    ]
    results = run_bass_kernel_spmd(nc, in_maps, core_ids=list(range(N_CORES)))

    rs = np.stack([r["out"] for r in results.results]).astype(np.float64)
    s = rs[:, :, :D].sum(axis=(0, 1))   # (256,) global column sums
    sumsq = 0.0                         # trace(sim)
    for i, kind in enumerate(nc._seg_kinds):
        col = rs[:, :, D + i]
        sumsq += col.sum() if kind == "full" else col[:, 0].sum()
    off_diag = float(s @ s) - sumsq
    loss = abs(off_diag / (M * (M - 1)))
    return np.float32(loss)
